# revision 1
# baseline (speedup 1.0000x reference)
"""AttentionRNNCell (streaming-softmax attention RNN) for 8 TRN2 NeuronCores.

kernel(x, kv_kernel, q_kernel) -> [B, T, D] float32

Math per (batch, head): kv = silu(x @ kv_kernel); s_t = <q_h, k_t>;
out_t = sum_h cumsum_t(v * e^s) / cumsum_t(e^s)   (unstabilized streaming
softmax — safe for this data distribution; |s| stays < ~8).

Strategy (data-parallel over batch, 4 batches/core):
  - K^T = Wk^T @ x^T on PE in [head*dim, t] layout (bf16 in, f32 psum),
    silu on ACT straight out of PSUM, s^T = Qblock^T @ silu(K^T) on PE,
    exp on ACT, den = cumsum via DVE tensor_tensor_scan along t,
    PE-transposes bring e/1/den back to [t, h] layout.
  - V = x^T.T @ Wv in [t, head*dim] layout, silu, ve = v*e (DVE broadcast
    multiply), cumsum over t via a column-rotated triangular-ones matmul
    (output row 0 = running total -> legal K=1 carry-broadcast source for
    the next chunk), out = sum_h num * (1/den) via DVE multiply + strided
    reduce. Output rows are un-rotated by the store DMAs.
"""

import numpy as np
from contextlib import ExitStack

import ml_dtypes

import bass_rust
import concourse.bass as bass
import concourse.mybir as mybir
import concourse.tile as tile
from concourse import bass_utils

AF = mybir.ActivationFunctionType
BF16 = mybir.dt.bfloat16
F32 = mybir.dt.float32
F32R = mybir.dt.float32r

P = 128
N_CORES = 8
B, T, I_DIM, H, D = 32, 1024, 1024, 16, 64
B_LOC = B // N_CORES


# ---------------------------------------------------------------------------
# TileContext patches: the walrus build in this container supports only ONE
# semaphore wait per instruction. (1) split the end-of-context drain's waits
# across several drains; (2) hoist extra scheduler-attached waits onto
# InstNoOp carriers just before the instruction on the same engine.
# ---------------------------------------------------------------------------

def _split_waits(self, inst):
    si = inst.sync_info
    if (
        si is not None
        and si.on_wait
        and len(si.on_wait) > 1
        and inst.engine != mybir.EngineType.Unassigned
    ):
        waits = list(si.on_wait)
        sem_waits = [w for w in waits if w.sync_type == "semaphore"]
        other = [w for w in waits if w.sync_type != "semaphore"]
        hoist = sem_waits[:-1] if sem_waits else []
        keep = sem_waits[-1:] + other if sem_waits else other
        if hoist:
            for w in hoist:
                nop = mybir.InstNoOp(
                    name=self.nc.get_next_instruction_name(),
                    sync_info=mybir.SyncInfo(on_wait=[w], on_update=[]),
                    bass_nofuse=True,
                    engine=inst.engine,
                )
                self.nc.register_instruction(nop, overwrite=True)
                self.nc.cur_bb.bb.add_instruction(nop)
            inst.sync_info = mybir.SyncInfo(
                on_wait=keep, on_update=list(si.on_update or [])
            )


def _patched_add_instruction(self, inst):
    _split_waits(self, inst)
    self.nc.register_instruction(inst, overwrite=True)
    self.nc.cur_bb.bb.add_instruction(inst)


def _patched_drain_and_barrier(self, tick_clock, wait_clock):
    nc = self.nc
    drain_inst = nc.sync.drain()
    wait_clock.add_sem_waits(
        drain_inst.ins, bass_rust.ScopedClock({None: tick_clock.global_clock})
    )
    si = drain_inst.ins.sync_info
    waits = list(si.on_wait) if si is not None and si.on_wait else []
    if len(waits) > 1:
        upds = list(si.on_update) if si.on_update else []
        drain_inst.ins.sync_info = bass_rust.SyncInfo(
            on_wait=[waits[0]], on_update=upds
        )
        for w in waits[1:]:
            extra = nc.sync.drain()
            extra.ins.sync_info = bass_rust.SyncInfo(on_wait=[w], on_update=[])

    nc.all_engine_barrier()
    assert self.sems is not None
    popped = nc._tile_sem_poison_stack.pop()
    assert popped is self._sem_poison
    nc.clear_and_free_semaphores(list(self.sems.allocated().values()))
    nc.all_engine_barrier()


def _apply_tile_patches():
    tile.TileContext._add_instruction = _patched_add_instruction
    tile.TileContext._drain_and_barrier = _patched_drain_and_barrier


# ---------------------------------------------------------------------------
# Kernel builder
# ---------------------------------------------------------------------------

def _mm_cast(ap):
    return ap.bitcast(F32R) if ap.dtype != F32R else ap


def _build(nc, tc, ctx):
    B_loc, T_, I, H_, D_ = B_LOC, T, I_DIM, H, D
    HD = H_ * D_
    NT = T_ // P
    KT = I // P
    NG = HD // P
    NB = HD // 512
    TC5 = T_ // 512

    xt_d = nc.dram_tensor("xt", [B_loc, I, T_], BF16, kind="ExternalInput").ap()
    wk_d = nc.dram_tensor("wk", [I, HD], BF16, kind="ExternalInput").ap()
    wv_d = nc.dram_tensor("wv", [I, HD], BF16, kind="ExternalInput").ap()
    qb_d = nc.dram_tensor("qb", [HD, H_], F32R, kind="ExternalInput").ap()
    u_d = nc.dram_tensor("u", [P, P], BF16, kind="ExternalInput").ap()
    ones_d = nc.dram_tensor("ones", [1, P], F32R, kind="ExternalInput").ap()
    id_d = nc.dram_tensor("ident", [P, P], F32, kind="ExternalInput").ap()
    out_d = nc.dram_tensor("out", [B_loc, T_, D_], F32, kind="ExternalOutput").ap()

    const = ctx.enter_context(tc.tile_pool(name="const", bufs=1))
    xt_pool = ctx.enter_context(tc.tile_pool(name="xt", bufs=2 * KT))
    ksil_pool = ctx.enter_context(tc.tile_pool(name="ksil", bufs=2))
    st_pool = ctx.enter_context(tc.tile_pool(name="st", bufs=3))
    epc_pool = ctx.enter_context(tc.tile_pool(name="epc", bufs=2 * NT))
    rden_pool = ctx.enter_context(tc.tile_pool(name="rden", bufs=2 * NT))
    vsil_pool = ctx.enter_context(tc.tile_pool(name="vsil", bufs=3))
    ve_pool = ctx.enter_context(tc.tile_pool(name="ve", bufs=2))
    cum_pool = ctx.enter_context(tc.tile_pool(name="cum", bufs=3))
    prod_pool = ctx.enter_context(tc.tile_pool(name="prod", bufs=2))
    o_pool = ctx.enter_context(tc.tile_pool(name="o", bufs=3))

    # PSUM: 8 banks. pa (3, shared tag) = K-path accumulators + transposes;
    # pv/pc 2 each -> 7 banks. (8/8 deadlocks the slot scheduler.)
    pa_pool = ctx.enter_context(tc.tile_pool(name="pa", bufs=3, space="PSUM"))
    pv_pool = ctx.enter_context(tc.tile_pool(name="pv", bufs=2, space="PSUM"))
    pc_pool = ctx.enter_context(tc.tile_pool(name="pc", bufs=2, space="PSUM"))

    # ---- weights/constants; wk and xt(b0) interleaved so the first K
    # accumulation group starts as soon as (wk[0], xt[0]) land ----
    wk_sb, wv_sb, qb_sb, xt_b0 = [], [], [], []
    for k in range(KT):
        t1 = const.tile([P, HD], BF16, tag=f"wk{k}")
        nc.sync.dma_start(t1[:], wk_d[k * P:(k + 1) * P, :])
        wk_sb.append(t1)
        t = xt_pool.tile([P, T_], BF16, tag="xt")
        nc.sync.dma_start(t[:, 0:512], xt_d[0, k * P:(k + 1) * P, 0:512])
        xt_b0.append(t)
    for k in range(KT):
        nc.sync.dma_start(xt_b0[k][:, 512:T_], xt_d[0, k * P:(k + 1) * P, 512:T_])
    for g in range(NG):
        t3 = const.tile([P, H_], F32R, tag=f"qb{g}")
        nc.sync.dma_start(t3[:], qb_d[g * P:(g + 1) * P, :])
        qb_sb.append(t3)
    u_sb = const.tile([P, P], BF16, tag="u")
    nc.sync.dma_start(u_sb[:], u_d[:])
    ones_sb = const.tile([1, P], F32R, tag="ones")
    nc.sync.dma_start(ones_sb[:], ones_d[:])
    id_sb = const.tile([P, P], F32, tag="ident")
    nc.sync.dma_start(id_sb[:], id_d[:])
    for k in range(KT):
        t2 = const.tile([P, HD], BF16, tag=f"wv{k}")
        nc.sync.dma_start(t2[:], wv_d[k * P:(k + 1) * P, :])
        wv_sb.append(t2)

    for b in range(B_loc):
        if b == 0:
            xt = xt_b0
        else:
            xt = []
            for k in range(KT):
                t = xt_pool.tile([P, T_], BF16, tag="xt")
                nc.sync.dma_start(t[:], xt_d[b, k * P:(k + 1) * P, :])
                xt.append(t)

        # ---- K path: s^T[h, t] ----
        sT = st_pool.tile([H_, T_], F32, tag="st")
        for tc5 in range(TC5):
            ps_s = pa_pool.tile([H_, 512], F32, tag="a")
            for g in range(NG):
                pk = pa_pool.tile([P, 512], F32, tag="a")
                for k in range(KT):
                    nc.tensor.matmul(
                        pk[:],
                        wk_sb[k][:, g * P:(g + 1) * P],
                        xt[k][:, tc5 * 512:(tc5 + 1) * 512],
                        start=(k == 0),
                        stop=(k == KT - 1),
                    )
                ksil = ksil_pool.tile([P, 512], F32R, tag="ksil")
                nc.scalar.activation(ksil[:], pk[:], AF.Silu)
                nc.tensor.matmul(
                    ps_s[:], qb_sb[g][:], ksil[:],
                    start=(g == 0), stop=(g == NG - 1),
                )
            nc.scalar.copy(sT[:, tc5 * 512:(tc5 + 1) * 512], ps_s[:])

        # e^T = exp(s^T); den^T = chained half-scans (emitted before the V
        # silus so exp + its act-table switch run first on ACT).
        eT = st_pool.tile([H_, T_], F32, tag="st")
        nc.scalar.activation(eT[:], sT[:], AF.Exp)
        denT = st_pool.tile([H_, T_], F32, tag="st")
        half = T_ // 2
        nc.vector.tensor_tensor_scan(
            denT[:, 0:half], eT[:, 0:half], eT[:, 0:half], 0.0,
            op0=mybir.AluOpType.add, op1=mybir.AluOpType.bypass,
        )
        nc.vector.tensor_tensor_scan(
            denT[:, half:T_], eT[:, half:T_], eT[:, half:T_],
            denT[:, half - 1:half],
            op0=mybir.AluOpType.add, op1=mybir.AluOpType.bypass,
        )

        # V projection + silu emitted PREFETCH chunks ahead: keeps PE busy
        # while the e-chain resolves.
        PREFETCH = 2

        def v_proj(c):
            vsil = vsil_pool.tile([P, HD], F32, tag="vsil")
            for nb in range(NB):
                pv = pv_pool.tile([P, 512], F32, tag="v")
                for k in range(KT):
                    nc.tensor.matmul(
                        pv[:],
                        xt[k][:, c * P:(c + 1) * P],
                        wv_sb[k][:, nb * 512:(nb + 1) * 512],
                        start=(k == 0),
                        stop=(k == KT - 1),
                    )
                nc.scalar.activation(vsil[:, nb * 512:(nb + 1) * 512], pv[:], AF.Silu)
            return vsil

        vsil_q = [v_proj(c) for c in range(min(PREFETCH, NT))]

        # transpose e^T / den^T into [t, h]; rden row-rotated by +1 to match
        # the rotated cumsum output (see below).
        e_c, rden_c = [], []
        for c in range(NT):
            pt_e = pa_pool.tile([P, H_], F32, tag="a")
            nc.tensor.transpose(pt_e[:], eT[:, c * P:(c + 1) * P], id_sb[:H_, :H_])
            ec = epc_pool.tile([P, H_], F32, tag="epc")
            nc.vector.tensor_copy(ec[:], pt_e[:])
            e_c.append(ec)
            pt_d = pa_pool.tile([P, H_], F32, tag="a")
            nc.tensor.transpose(pt_d[:], denT[:, c * P:(c + 1) * P], id_sb[:H_, :H_])
            rc = rden_pool.tile([P, H_], F32, tag="rden")
            nc.vector.reciprocal(rc[:], pt_d[:])
            rs = rden_pool.tile([P, H_], F32, tag="rdens")
            nc.gpsimd.dma_start(rs[0:1, :], rc[P - 1:P, :])
            nc.gpsimd.dma_start(rs[1:P, :], rc[0:P - 1, :])
            rden_c.append(rs)

        # ---- V path with rotated running cumsum ----
        # Ushift columns: out row 0 = chunk total (+carry) = inclusive prefix
        # at t=P-1; row m>=1 = inclusive prefix at t=m-1. Row 0 is the legal
        # (base-partition-0) carry source for the next chunk's K=1 broadcast
        # matmul. The store DMAs un-rotate the rows.
        prev_cum = None
        for c in range(NT):
            vsil = vsil_q[c]
            if c + PREFETCH < NT:
                vsil_q.append(v_proj(c + PREFETCH))

            ve = ve_pool.tile([P, HD], BF16, tag="ve")
            e_bc = e_c[c][:].unsqueeze(2).broadcast_to((P, H_, D_))
            nc.vector.tensor_mul(
                ve[:].rearrange("p (h d) -> p h d", h=H_),
                vsil[:].rearrange("p (h d) -> p h d", h=H_),
                e_bc,
            )

            cum = cum_pool.tile([P, HD], F32R, tag="cum")
            pcs = []
            for nb in range(NB):
                pc = pc_pool.tile([P, 512], F32, tag="c")
                nc.tensor.matmul(
                    pc[:], u_sb[:], ve[:, nb * 512:(nb + 1) * 512],
                    start=True, stop=(c == 0),
                )
                pcs.append(pc)
            if c > 0:
                for nb in range(NB):
                    nc.tensor.matmul(
                        pcs[nb][:], ones_sb[:],
                        prev_cum[0:1, nb * 512:(nb + 1) * 512],
                        start=False, stop=True,
                    )
            for nb in range(NB):
                nc.scalar.copy(cum[:, nb * 512:(nb + 1) * 512], pcs[nb][:])
            prev_cum = cum

            prod = prod_pool.tile([P, HD], F32, tag="prod")
            r_bc = rden_c[c][:].unsqueeze(2).broadcast_to((P, H_, D_))
            nc.vector.tensor_mul(
                prod[:].rearrange("p (h d) -> p h d", h=H_),
                cum[:].bitcast(F32).rearrange("p (h d) -> p h d", h=H_),
                r_bc,
            )
            o = o_pool.tile([P, D_], F32, tag="o")
            nc.vector.reduce_sum(
                o[:], prod[:].rearrange("p (h d) -> p d h", h=H_),
                axis=mybir.AxisListType.X,
            )
            nc.gpsimd.dma_start(out_d[b, c * P + P - 1:c * P + P, :], o[0:1, :])
            nc.gpsimd.dma_start(out_d[b, c * P:(c + 1) * P - 1, :], o[1:P, :])


_NC_CACHE = []


def _build_nc():
    if _NC_CACHE:
        return _NC_CACHE[0]
    _apply_tile_patches()
    nc = bass.Bass(trn_type="TRN2", target_bir_lowering=False, debug=False)
    with tile.TileContext(nc) as tc:
        with ExitStack() as ctx:
            _build(nc, tc, ctx)
    _NC_CACHE.append(nc)
    return nc


def _host_prep(x_shard, wk, wv, shared):
    xt = np.ascontiguousarray(x_shard.transpose(0, 2, 1)).astype(ml_dtypes.bfloat16)
    m = dict(shared)
    m["xt"] = xt
    return m


def kernel(x, kv_kernel, q_kernel):
    x = np.asarray(x, dtype=np.float32)
    kv_kernel = np.asarray(kv_kernel, dtype=np.float32)
    q_kernel = np.asarray(q_kernel, dtype=np.float32)
    HD = H * D

    wk = np.ascontiguousarray(kv_kernel[..., 0].reshape(I_DIM, HD))
    wv = np.ascontiguousarray(kv_kernel[..., 1].reshape(I_DIM, HD))
    qb = np.zeros((HD, H), dtype=np.float32)
    for h in range(H):
        qb[h * D:(h + 1) * D, h] = q_kernel[h]
    u = np.triu(np.ones((P, P), dtype=np.float32), k=1)
    u[:, 0] = 1.0
    shared = {
        "wk": wk.astype(ml_dtypes.bfloat16),
        "wv": wv.astype(ml_dtypes.bfloat16),
        "qb": qb,
        "u": u.astype(ml_dtypes.bfloat16),
        "ones": np.ones((1, P), dtype=np.float32),
        "ident": np.eye(P, dtype=np.float32),
    }

    nc = _build_nc()
    in_maps = [
        _host_prep(x[c * B_LOC:(c + 1) * B_LOC], wk, wv, shared)
        for c in range(N_CORES)
    ]
    res = bass_utils.run_bass_kernel_spmd(nc, in_maps, core_ids=list(range(N_CORES)))
    out = np.concatenate([r["out"] for r in res.results], axis=0)
    return out.astype(np.float32)



# revision 12
# speedup vs baseline: 1.1516x; 1.1516x over previous
"""AttentionRNNCell (streaming-softmax attention RNN) for 8 TRN2 NeuronCores.

kernel(x, kv_kernel, q_kernel) -> [B, T, D] float32

Math per (batch, head): kv = silu(x @ kv_kernel); s_t = <q_h, k_t>;
out_t = sum_h cumsum_t(v * e^s) / cumsum_t(e^s)   (unstabilized streaming
softmax — safe for this data distribution; |s| stays < ~8).

Strategy (data-parallel over batch, 4 batches/core):
  - K path: fp8e4 DoubleRow projection (weights pre-scaled x32, un-scaled in
    the silu's ACT scale), s^T = Qblock^T @ silu(K^T) on PE in [h, t] layout,
    exp on ACT straight out of PSUM, den^T = chained DVE half-scans, 1/den
    with a +1-rotated output AP (aligns with the rotated cumsum below), PE
    transposes bring e^T / rden^T to [t, h].
  - V path: [t, hd] projection with d-major head layout; time-chunk 0 in
    bf16, chunks 1..7 in fp8 DoubleRow (early outputs average few v terms, so
    fp8 noise there would break tolerance; later chunks average it away).
    ve = v*e on GpSimd (bf16), cumsum over t via column-rotated
    triangular-ones matmul (output row 0 = running total -> legal
    base-partition-0 carry for the next chunk's K=1 broadcast matmul),
    prod = cum * (1/den) read straight from PSUM on DVE, head-sum is a
    contiguous stride-1 reduce (d-major), store un-rotates via 2 HW DMAs.
  - Cross-batch software pipelining: batch b+1's K-projection groups are
    emitted between batch b's V chunks so the PE never idles at batch
    boundaries (keeps the HAM clock gate warm).
"""

import numpy as np
from contextlib import ExitStack

import ml_dtypes

import bass_rust
import concourse.bass as bass
import concourse.mybir as mybir
import concourse.tile as tile
from concourse import bass_utils

AF = mybir.ActivationFunctionType
BF16 = mybir.dt.bfloat16
F32 = mybir.dt.float32
F32R = mybir.dt.float32r
FP8 = mybir.dt.float8e4
DR = mybir.MatmulPerfMode.DoubleRow

P = 128
N_CORES = 8
B, T, I_DIM, H, D = 32, 1024, 1024, 16, 64
B_LOC = B // N_CORES
HD = H * D
KT = I_DIM // P          # 8 contraction tiles
NT = T // P              # 8 time chunks
NG = HD // P             # 8 output groups (K path)
NB = HD // 512           # 2 psum-width groups (V path)
TC5 = T // 512           # 2 time-half groups (K path)
SC = 32.0                # fp8 weight pre-scale


# ---------------------------------------------------------------------------
# TileContext patches: the walrus build in this container supports only ONE
# semaphore wait per instruction. (1) split the end-of-context drain's waits
# across several drains; (2) hoist extra scheduler-attached waits onto
# InstNoOp carriers just before the instruction on the same engine.
# ---------------------------------------------------------------------------

def _split_waits(self, inst):
    si = inst.sync_info
    if (
        si is not None
        and si.on_wait
        and len(si.on_wait) > 1
        and inst.engine != mybir.EngineType.Unassigned
    ):
        waits = list(si.on_wait)
        sem_waits = [w for w in waits if w.sync_type == "semaphore"]
        other = [w for w in waits if w.sync_type != "semaphore"]
        hoist = sem_waits[:-1] if sem_waits else []
        keep = sem_waits[-1:] + other if sem_waits else other
        if hoist:
            for w in hoist:
                nop = mybir.InstNoOp(
                    name=self.nc.get_next_instruction_name(),
                    sync_info=mybir.SyncInfo(on_wait=[w], on_update=[]),
                    bass_nofuse=True,
                    engine=inst.engine,
                )
                self.nc.register_instruction(nop, overwrite=True)
                self.nc.cur_bb.bb.add_instruction(nop)
            inst.sync_info = mybir.SyncInfo(
                on_wait=keep, on_update=list(si.on_update or [])
            )


def _patched_add_instruction(self, inst):
    _split_waits(self, inst)
    self.nc.register_instruction(inst, overwrite=True)
    self.nc.cur_bb.bb.add_instruction(inst)


def _patched_drain_and_barrier(self, tick_clock, wait_clock):
    nc = self.nc
    drain_inst = nc.sync.drain()
    wait_clock.add_sem_waits(
        drain_inst.ins, bass_rust.ScopedClock({None: tick_clock.global_clock})
    )
    si = drain_inst.ins.sync_info
    waits = list(si.on_wait) if si is not None and si.on_wait else []
    if len(waits) > 1:
        upds = list(si.on_update) if si.on_update else []
        drain_inst.ins.sync_info = bass_rust.SyncInfo(
            on_wait=[waits[0]], on_update=upds
        )
        for w in waits[1:]:
            extra = nc.sync.drain()
            extra.ins.sync_info = bass_rust.SyncInfo(on_wait=[w], on_update=[])

    nc.all_engine_barrier()
    assert self.sems is not None
    popped = nc._tile_sem_poison_stack.pop()
    assert popped is self._sem_poison
    nc.clear_and_free_semaphores(list(self.sems.allocated().values()))
    nc.all_engine_barrier()


def _apply_tile_patches():
    tile.TileContext._add_instruction = _patched_add_instruction
    tile.TileContext._drain_and_barrier = _patched_drain_and_barrier


# ---------------------------------------------------------------------------
# Kernel builder
# ---------------------------------------------------------------------------

class _Builder:
    def __init__(self, nc, tc, ctx):
        self.nc = nc
        self.tc = tc

        self.xt8_d = nc.dram_tensor("xt8", [B_LOC, P, KT, T], FP8, kind="ExternalInput").ap()
        self.xbf_d = nc.dram_tensor("xbf", [B_LOC, P, KT, P], BF16, kind="ExternalInput").ap()
        self.wk8_d = nc.dram_tensor("wk8", [P, KT, HD], FP8, kind="ExternalInput").ap()
        self.wv8_d = nc.dram_tensor("wv8", [P, KT, HD], FP8, kind="ExternalInput").ap()
        self.wvb_d = nc.dram_tensor("wvb", [P, KT, HD], BF16, kind="ExternalInput").ap()
        self.qbt_d = nc.dram_tensor("qbt", [P, NG, H], BF16, kind="ExternalInput").ap()
        self.u_d = nc.dram_tensor("u", [P, P], BF16, kind="ExternalInput").ap()
        self.ones_d = nc.dram_tensor("ones", [1, P], F32R, kind="ExternalInput").ap()
        self.idb_d = nc.dram_tensor("idb", [H, H], BF16, kind="ExternalInput").ap()
        self.idf_d = nc.dram_tensor("idf", [H, H], F32, kind="ExternalInput").ap()
        self.out_d = nc.dram_tensor("out", [B_LOC, T, D], F32, kind="ExternalOutput").ap()

        ep = ctx.enter_context
        self.const = ep(tc.tile_pool(name="const", bufs=1))
        self.xt_pool = ep(tc.tile_pool(name="xt", bufs=B_LOC))
        self.xb_pool = ep(tc.tile_pool(name="xb", bufs=B_LOC))
        self.ksil_pool = ep(tc.tile_pool(name="ksil", bufs=3))
        self.et_pool = ep(tc.tile_pool(name="et", bufs=2))
        self.dn_pool = ep(tc.tile_pool(name="dn", bufs=2))
        self.rd_pool = ep(tc.tile_pool(name="rd", bufs=2))
        self.ec_pool = ep(tc.tile_pool(name="ec", bufs=2 * NT))
        self.rc_pool = ep(tc.tile_pool(name="rc", bufs=2 * NT))
        self.vsil_pool = ep(tc.tile_pool(name="vsil", bufs=2))
        self.ve_pool = ep(tc.tile_pool(name="ve", bufs=2))
        self.crow_pool = ep(tc.tile_pool(name="crow", bufs=3))
        self.prod_pool = ep(tc.tile_pool(name="prod", bufs=2))
        self.o_pool = ep(tc.tile_pool(name="o", bufs=2))

        # PSUM: 7 usable banks: proj/transpose 2 + ps_s 2 + cum 3
        self.proj_pool = ep(tc.tile_pool(name="pj", bufs=2, space="PSUM"))
        self.ss_pool = ep(tc.tile_pool(name="ss", bufs=2, space="PSUM"))
        self.pc_pool = ep(tc.tile_pool(name="pc", bufs=3, space="PSUM"))
        self.pt_pool = self.proj_pool

        # per-batch live state
        self.xt8 = [None] * B_LOC
        self.xbf = [None] * B_LOC
        self.ps_s = [None] * B_LOC
        self.eT = [None] * B_LOC
        self.rdenT = [None] * B_LOC
        self.e_c = [[None] * NT for _ in range(B_LOC)]
        self.r_c = [[None] * NT for _ in range(B_LOC)]
        self.crow = [[None] * NT for _ in range(B_LOC)]
        self.o_all = [None] * B_LOC
        self.s_mm_queue = []  # delayed s-matmuls: (b, tc5, g, ksil)

    # ---- input loads ----
    def load_weights(self):
        nc = self.nc
        self.wk8 = self.const.tile([P, KT, HD], FP8, tag="wk8")
        # two halves on two queues so the first K group starts ASAP
        nc.sync.dma_start(self.wk8[:, 0:KT // 2, :], self.wk8_d[:, 0:KT // 2, :])
        nc.scalar.dma_start(self.wk8[:, KT // 2:KT, :], self.wk8_d[:, KT // 2:KT, :])
        self.load_x(0, split=True)
        self.qbt = self.const.tile([P, NG, H], BF16, tag="qbt")
        nc.scalar.dma_start(self.qbt[:], self.qbt_d[:])
        self.u_sb = self.const.tile([P, P], BF16, tag="u")
        nc.scalar.dma_start(self.u_sb[:], self.u_d[:])
        self.ones_sb = self.const.tile([1, P], F32R, tag="ones")
        nc.scalar.dma_start(self.ones_sb[:], self.ones_d[:])
        self.idb = self.const.tile([H, H], BF16, tag="idb")
        nc.scalar.dma_start(self.idb[:], self.idb_d[:])
        self.idf = self.const.tile([H, H], F32, tag="idf")
        nc.scalar.dma_start(self.idf[:], self.idf_d[:])
        self.wvb = self.const.tile([P, KT, HD], BF16, tag="wvb")
        nc.scalar.dma_start(self.wvb[:], self.wvb_d[:])
        self.wv8 = self.const.tile([P, KT, HD], FP8, tag="wv8")
        nc.sync.dma_start(self.wv8[:], self.wv8_d[:])
        for b in range(1, B_LOC):
            self.load_x(b)

    def load_x(self, b, split=False):
        nc = self.nc
        t = self.xt_pool.tile([P, KT, T], FP8, tag="xt8")
        if split:
            nc.sync.dma_start(t[:, :, 0:512], self.xt8_d[b, :, :, 0:512])
            nc.sync.dma_start(t[:, :, 512:T], self.xt8_d[b, :, :, 512:T])
        else:
            nc.sync.dma_start(t[:], self.xt8_d[b])
        self.xt8[b] = t
        tb = self.xb_pool.tile([P, KT, P], BF16, tag="xbf")
        nc.sync.dma_start(tb[:], self.xbf_d[b])
        self.xbf[b] = tb

    # ---- K path ----
    def emit_k_group(self, b, tc5, g):
        """fp8 DoubleRow projection group + silu; s-matmul is queued (1-delay)."""
        nc = self.nc
        if self.ps_s[b] is None:
            self.ps_s[b] = [
                self.ss_pool.tile([H, 512], F32, tag="ss", name=f"ss{i}")
                for i in range(TC5)
            ]
        pk = self.proj_pool.tile([P, 512], F32, tag="proj")
        for kk in range(KT // 2):
            nc.tensor.matmul(
                pk[:],
                self.wk8[:, 2 * kk:2 * kk + 2, g * P:(g + 1) * P],
                self.xt8[b][:, 2 * kk:2 * kk + 2, tc5 * 512:(tc5 + 1) * 512],
                start=(kk == 0),
                stop=(kk == KT // 2 - 1),
                perf_mode=DR,
            )
        ksil = self.ksil_pool.tile([P, 512], BF16, tag="ksil")
        nc.scalar.activation(ksil[:], pk[:], AF.Silu, scale=1.0 / SC)
        self.s_mm_queue.append((b, tc5, g, ksil))
        if len(self.s_mm_queue) > 1:
            self.flush_s_mm(1)

    def flush_s_mm(self, keep=0):
        nc = self.nc
        while len(self.s_mm_queue) > keep:
            b, tc5, g, ksil = self.s_mm_queue.pop(0)
            nc.tensor.matmul(
                self.ps_s[b][tc5][:],
                self.qbt[:, g, :],
                ksil[:],
                start=(g == 0),
                stop=(g == NG - 1),
            )

    def emit_k_post(self, b):
        """exp -> den scan chain -> rotated reciprocal (ACT + DVE only)."""
        nc = self.nc
        eT = self.et_pool.tile([H, T], BF16, tag="et")
        for tc5 in range(TC5):
            nc.scalar.activation(
                eT[:, tc5 * 512:(tc5 + 1) * 512], self.ps_s[b][tc5][:], AF.Exp
            )
        denT = self.dn_pool.tile([H, T], F32, tag="dn")
        half = T // 2
        nc.vector.tensor_tensor_scan(
            denT[:, 0:half], eT[:, 0:half], eT[:, 0:half], 0.0,
            op0=mybir.AluOpType.add, op1=mybir.AluOpType.bypass,
        )
        nc.vector.tensor_tensor_scan(
            denT[:, half:T], eT[:, half:T], eT[:, half:T],
            denT[:, half - 1:half],
            op0=mybir.AluOpType.add, op1=mybir.AluOpType.bypass,
        )
        # 1/den with output rotated +1 along t within each chunk: row m of the
        # later [t, h] transpose must hold 1/den at t=m-1 (row 0 -> t=P-1) to
        # match the rotated cumsum layout.
        rdenT = self.rd_pool.tile([H, NT, P], F32, tag="rd")
        dv = denT[:].rearrange("h (c m) -> h c m", c=NT)
        nc.vector.reciprocal(rdenT[:, :, 1:P], dv[:, :, 0:P - 1])
        nc.vector.reciprocal(rdenT[:, :, 0:1], dv[:, :, P - 1:P])
        self.eT[b] = eT
        self.rdenT[b] = rdenT
        self.ps_s[b] = None

    def emit_transpose_pair(self, b, c):
        """Transpose e^T / rden^T for chunks c and c+1 (paired per PSUM tile)."""
        nc = self.nc
        pt_e = self.pt_pool.tile([P, 2, H], BF16, tag="proj")
        for j in range(2):
            nc.tensor.transpose(
                pt_e[:, j, :], self.eT[b][:, (c + j) * P:(c + j + 1) * P], self.idb[:]
            )
        ec = self.ec_pool.tile([P, 2, H], BF16, tag="ec")
        nc.vector.tensor_copy(ec[:], pt_e[:])
        pt_d = self.pt_pool.tile([P, 2, H], F32, tag="proj")
        for j in range(2):
            nc.tensor.transpose(
                pt_d[:, j, :], self.rdenT[b][:, c + j, :], self.idf[:]
            )
        rc = self.rc_pool.tile([P, 2, H], F32, tag="rc")
        nc.vector.tensor_copy(rc[:], pt_d[:])
        for j in range(2):
            self.e_c[b][c + j] = ec[:, j, :]
            self.r_c[b][c + j] = rc[:, j, :]

    # ---- V path ----
    def emit_v_proj(self, b, c):
        nc = self.nc
        vsil = self.vsil_pool.tile([P, HD], BF16, tag="vsil")
        for nb in range(NB):
            pv = self.proj_pool.tile([P, 512], F32, tag="proj")
            if c == 0:
                # bf16 chunk: fp8 noise on the first time chunk would exceed
                # tolerance (few terms averaged in the streaming softmax yet)
                for k in range(KT):
                    nc.tensor.matmul(
                        pv[:],
                        self.xbf[b][:, k, :],
                        self.wvb[:, k, nb * 512:(nb + 1) * 512],
                        start=(k == 0),
                        stop=(k == KT - 1),
                    )
                nc.scalar.activation(vsil[:, nb * 512:(nb + 1) * 512], pv[:], AF.Silu)
            else:
                for kk in range(KT // 2):
                    nc.tensor.matmul(
                        pv[:],
                        self.xt8[b][:, 2 * kk:2 * kk + 2, c * P:(c + 1) * P],
                        self.wv8[:, 2 * kk:2 * kk + 2, nb * 512:(nb + 1) * 512],
                        start=(kk == 0),
                        stop=(kk == KT // 2 - 1),
                        perf_mode=DR,
                    )
                nc.scalar.activation(
                    vsil[:, nb * 512:(nb + 1) * 512], pv[:], AF.Silu, scale=1.0 / SC
                )
        return vsil

    def emit_v_tail(self, b, c, vsil):
        nc = self.nc
        # ve = v * e (d-major: [p, d, h]; e broadcast over d) on GpSimd
        ve = self.ve_pool.tile([P, HD], BF16, tag="ve")
        e_bc = self.e_c[b][c].unsqueeze(1).broadcast_to((P, D, H))
        nc.gpsimd.tensor_mul(
            ve[:].rearrange("p (d h) -> p d h", h=H),
            vsil[:].rearrange("p (d h) -> p d h", h=H),
            e_bc,
        )
        # rotated running cumsum over t via triangular-ones matmul + K=1 carry
        pcs = []
        for nb in range(NB):
            pc = self.pc_pool.tile([P, 512], F32, tag="pc")
            nc.tensor.matmul(
                pc[:], self.u_sb[:], ve[:, nb * 512:(nb + 1) * 512],
                start=True, stop=(c == 0),
            )
            if c > 0:
                nc.tensor.matmul(
                    pc[:], self.ones_sb[:],
                    self.crow[b][c - 1][:, nb, :],
                    start=False, stop=True,
                )
            pcs.append(pc)
        crow = self.crow_pool.tile([1, NB, 512], F32R, tag="crow")
        for nb in range(NB):
            nc.scalar.copy(crow[:, nb, :], pcs[nb][0:1, :])
        self.crow[b][c] = crow
        # prod = cum * rden (read cum straight from PSUM), then head-sum
        if self.o_all[b] is None:
            self.o_all[b] = self.o_pool.tile([P, NT, D], F32, tag="o", name="o")
        prod = self.prod_pool.tile([P, HD], F32, tag="prod")
        r_bc = self.r_c[b][c].unsqueeze(1).broadcast_to((P, D // NB, H))
        for nb in range(NB):
            nc.vector.tensor_mul(
                prod[:, nb * 512:(nb + 1) * 512].rearrange("p (d h) -> p d h", h=H),
                pcs[nb][:].rearrange("p (d h) -> p d h", h=H),
                r_bc,
            )
        nc.vector.reduce_sum(
            self.o_all[b][:, c, :],
            prod[:].rearrange("p (d h) -> p d h", h=H),
            axis=mybir.AxisListType.X,
        )

    def emit_store(self, b):
        nc = self.nc
        o = self.o_all[b]
        dst = self.out_d[b].rearrange("(c m) d -> m c d", m=P)
        nc.sync.dma_start(dst[0:P - 1], o[1:P])
        nc.sync.dma_start(dst[P - 1:P], o[0:1])
        self.o_all[b] = None

    # ---- top level ----
    def build(self):
        self.load_weights()
        # batch 0 K phase (prologue, nothing to interleave into)
        for tc5 in range(TC5):
            for g in range(NG):
                self.emit_k_group(0, tc5, g)
        self.flush_s_mm()
        self.emit_k_post(0)

        for b in range(B_LOC):
            kq = (
                [(tc5, g) for tc5 in range(TC5) for g in range(NG)]
                if b + 1 < B_LOC else []
            )
            # front-load next batch's K groups: 3 per chunk then the rest
            kq_sched = [3, 3, 3, 3, 3, 1, 0, 0]
            for c in range(NT):
                vsil = self.emit_v_proj(b, c)
                if c < 4:
                    self.emit_transpose_pair(b, 2 * c)
                for _ in range(kq_sched[c]):
                    if kq:
                        tc5, g = kq.pop(0)
                        self.emit_k_group(b + 1, tc5, g)
                self.emit_v_tail(b, c, vsil)
                if c == 5 and b + 1 < B_LOC:
                    self.flush_s_mm()
                    self.emit_k_post(b + 1)
            self.emit_store(b)


def _build(nc, tc, ctx):
    _Builder(nc, tc, ctx).build()


_NC_CACHE = []


def _build_nc():
    if _NC_CACHE:
        return _NC_CACHE[0]
    _apply_tile_patches()
    nc = bass.Bass(trn_type="TRN2", target_bir_lowering=False, debug=False)
    with tile.TileContext(nc) as tc:
        with ExitStack() as ctx:
            _build(nc, tc, ctx)
    _NC_CACHE.append(nc)
    return nc


def _host_prep(x_shard, shared):
    # xt8[b, p, k, t] = x[b, t, k*128+p] as fp8
    xt = np.ascontiguousarray(x_shard.transpose(0, 2, 1))  # [B_loc, I, T]
    xt8 = xt.reshape(B_LOC, KT, P, T).transpose(0, 2, 1, 3)  # [B_loc, P, KT, T]
    m = dict(shared)
    m["xt8"] = np.ascontiguousarray(xt8).astype(ml_dtypes.float8_e4m3fn)
    m["xbf"] = np.ascontiguousarray(xt8[:, :, :, 0:P]).astype(ml_dtypes.bfloat16)
    return m


def kernel(x, kv_kernel, q_kernel):
    x = np.asarray(x, dtype=np.float32)
    kv_kernel = np.asarray(kv_kernel, dtype=np.float32)
    q_kernel = np.asarray(q_kernel, dtype=np.float32)

    wk = kv_kernel[..., 0].reshape(I_DIM, HD)
    wv = kv_kernel[..., 1].reshape(I_DIM, HD)
    # d-major column order for the V path (head-sum becomes stride-1 reduce)
    wv_dm = wv.reshape(I_DIM, H, D).transpose(0, 2, 1).reshape(I_DIM, HD)

    def to_ktile(w):  # [I, HD] -> [P, KT, HD]
        return np.ascontiguousarray(w.reshape(KT, P, HD).transpose(1, 0, 2))

    qbt = np.zeros((P, NG, H), dtype=np.float32)
    for h in range(H):
        g, r = divmod(h * D, P)
        qbt[r:r + D, g, h] = q_kernel[h]
    u = np.triu(np.ones((P, P), dtype=np.float32), k=1)
    u[:, 0] = 1.0
    shared = {
        "wk8": to_ktile(wk * SC).astype(ml_dtypes.float8_e4m3fn),
        "wv8": to_ktile(wv_dm * SC).astype(ml_dtypes.float8_e4m3fn),
        "wvb": to_ktile(wv_dm).astype(ml_dtypes.bfloat16),
        "qbt": qbt.astype(ml_dtypes.bfloat16),
        "u": u.astype(ml_dtypes.bfloat16),
        "ones": np.ones((1, P), dtype=np.float32),
        "idb": np.eye(H, dtype=np.float32).astype(ml_dtypes.bfloat16),
        "idf": np.eye(H, dtype=np.float32),
    }

    nc = _build_nc()
    in_maps = [
        _host_prep(x[c * B_LOC:(c + 1) * B_LOC], shared)
        for c in range(N_CORES)
    ]
    res = bass_utils.run_bass_kernel_spmd(nc, in_maps, core_ids=list(range(N_CORES)))
    out = np.concatenate([r["out"] for r in res.results], axis=0)
    return out.astype(np.float32)


# revision 18
# speedup vs baseline: 1.3656x; 1.1859x over previous
"""AttentionRNNCell (streaming-softmax attention RNN) for 8 TRN2 NeuronCores.

kernel(x, kv_kernel, q_kernel) -> [B, T, D] float32

Math per (batch, head): kv = silu(x @ kv_kernel); s_t = <q_h, k_t>;
out_t = sum_h cumsum_t(v * e^s) / cumsum_t(e^s)   (unstabilized streaming
softmax — safe for this data distribution; |s| stays < ~8).

Strategy (data-parallel over batch, 4 batches/core):
  - K path: fp8e4 DoubleRow projection (weights pre-scaled x32, un-scaled in
    the silu's ACT scale), s^T = Qblock^T @ silu(K^T) on PE in [h, t] layout,
    exp on ACT straight out of PSUM, den^T = chained DVE half-scans, 1/den
    with a +1-rotated output AP (aligns with the rotated cumsum below), PE
    transposes bring e^T / rden^T to [t, h].
  - V path: [t, hd] projection with d-major head layout; time-chunk 0 in
    bf16, chunks 1..7 in fp8 DoubleRow (early outputs average few v terms, so
    fp8 noise there would break tolerance; later chunks average it away).
    ve = v*e on GpSimd (bf16), cumsum over t via column-rotated
    triangular-ones matmul (output row 0 = running total -> legal
    base-partition-0 carry for the next chunk's K=1 broadcast matmul),
    prod = cum * (1/den) read straight from PSUM on DVE, head-sum is a
    contiguous stride-1 reduce (d-major), store un-rotates via 2 HW DMAs.
  - Cross-batch software pipelining: batch b+1's K-projection groups are
    emitted between batch b's V chunks so the PE never idles at batch
    boundaries (keeps the HAM clock gate warm).
"""

import numpy as np
from contextlib import ExitStack

import ml_dtypes

import bass_rust
import concourse.bass as bass
import concourse.mybir as mybir
import concourse.tile as tile
from concourse import bass_utils

AF = mybir.ActivationFunctionType
BF16 = mybir.dt.bfloat16
F32 = mybir.dt.float32
F32R = mybir.dt.float32r
FP8 = mybir.dt.float8e4
DR = mybir.MatmulPerfMode.DoubleRow

P = 128
N_CORES = 8
B, T, I_DIM, H, D = 32, 1024, 1024, 16, 64
B_LOC = B // N_CORES
HD = H * D
KT = I_DIM // P          # 8 contraction tiles
NT = T // P              # 8 time chunks
NG = HD // P             # 8 output groups (K path)
NB = HD // 512           # 2 psum-width groups (V path)
TC5 = T // 512           # 2 time-half groups (K path)
SC = 32.0                # fp8 weight pre-scale


# ---------------------------------------------------------------------------
# TileContext patches: the walrus build in this container supports only ONE
# semaphore wait per instruction. (1) split the end-of-context drain's waits
# across several drains; (2) hoist extra scheduler-attached waits onto
# InstNoOp carriers just before the instruction on the same engine.
# ---------------------------------------------------------------------------

def _split_waits(self, inst):
    si = inst.sync_info
    if (
        si is not None
        and si.on_wait
        and len(si.on_wait) > 1
        and inst.engine != mybir.EngineType.Unassigned
    ):
        waits = list(si.on_wait)
        sem_waits = [w for w in waits if w.sync_type == "semaphore"]
        other = [w for w in waits if w.sync_type != "semaphore"]
        hoist = sem_waits[:-1] if sem_waits else []
        keep = sem_waits[-1:] + other if sem_waits else other
        if hoist:
            for w in hoist:
                nop = mybir.InstNoOp(
                    name=self.nc.get_next_instruction_name(),
                    sync_info=mybir.SyncInfo(on_wait=[w], on_update=[]),
                    bass_nofuse=True,
                    engine=inst.engine,
                )
                self.nc.register_instruction(nop, overwrite=True)
                self.nc.cur_bb.bb.add_instruction(nop)
            inst.sync_info = mybir.SyncInfo(
                on_wait=keep, on_update=list(si.on_update or [])
            )


def _patched_add_instruction(self, inst):
    _split_waits(self, inst)
    self.nc.register_instruction(inst, overwrite=True)
    self.nc.cur_bb.bb.add_instruction(inst)


def _patched_drain_and_barrier(self, tick_clock, wait_clock):
    nc = self.nc
    drain_inst = nc.sync.drain()
    wait_clock.add_sem_waits(
        drain_inst.ins, bass_rust.ScopedClock({None: tick_clock.global_clock})
    )
    si = drain_inst.ins.sync_info
    waits = list(si.on_wait) if si is not None and si.on_wait else []
    if len(waits) > 1:
        upds = list(si.on_update) if si.on_update else []
        drain_inst.ins.sync_info = bass_rust.SyncInfo(
            on_wait=[waits[0]], on_update=upds
        )
        for w in waits[1:]:
            extra = nc.sync.drain()
            extra.ins.sync_info = bass_rust.SyncInfo(on_wait=[w], on_update=[])

    nc.all_engine_barrier()
    assert self.sems is not None
    popped = nc._tile_sem_poison_stack.pop()
    assert popped is self._sem_poison
    nc.clear_and_free_semaphores(list(self.sems.allocated().values()))
    nc.all_engine_barrier()


def _apply_tile_patches():
    tile.TileContext._add_instruction = _patched_add_instruction
    tile.TileContext._drain_and_barrier = _patched_drain_and_barrier


# ---------------------------------------------------------------------------
# Kernel builder
# ---------------------------------------------------------------------------

class _Builder:
    def __init__(self, nc, tc, ctx):
        self.nc = nc
        self.tc = tc

        self.xt8_d = nc.dram_tensor("xt8", [B_LOC, P, KT, T], FP8, kind="ExternalInput").ap()
        self.xbf_d = nc.dram_tensor("xbf", [B_LOC, P, KT, P], BF16, kind="ExternalInput").ap()
        self.wk8_d = nc.dram_tensor("wk8", [P, KT, HD], FP8, kind="ExternalInput").ap()
        self.wv8_d = nc.dram_tensor("wv8", [P, KT, HD], FP8, kind="ExternalInput").ap()
        self.wvb_d = nc.dram_tensor("wvb", [P, KT, HD], BF16, kind="ExternalInput").ap()
        self.qbt_d = nc.dram_tensor("qbt", [P, NG, H], BF16, kind="ExternalInput").ap()
        self.u_d = nc.dram_tensor("u", [P, P], BF16, kind="ExternalInput").ap()
        self.ones_d = nc.dram_tensor("ones", [1, P], F32R, kind="ExternalInput").ap()
        self.idb_d = nc.dram_tensor("idb", [H, H], BF16, kind="ExternalInput").ap()
        self.idf_d = nc.dram_tensor("idf", [H, H], F32, kind="ExternalInput").ap()
        self.out_d = nc.dram_tensor("out", [B_LOC, T, D], F32, kind="ExternalOutput").ap()

        ep = ctx.enter_context
        self.const = ep(tc.tile_pool(name="const", bufs=1))
        self.xt_pool = ep(tc.tile_pool(name="xt", bufs=B_LOC))
        self.xb_pool = ep(tc.tile_pool(name="xb", bufs=B_LOC))
        self.ksil_pool = ep(tc.tile_pool(name="ksil", bufs=3))
        self.et_pool = ep(tc.tile_pool(name="et", bufs=2))
        self.dn_pool = ep(tc.tile_pool(name="dn", bufs=2))
        self.rd_pool = ep(tc.tile_pool(name="rd", bufs=2))
        self.ec_pool = ep(tc.tile_pool(name="ec", bufs=2 * NT))
        self.rc_pool = ep(tc.tile_pool(name="rc", bufs=2 * NT))
        self.vsil_pool = ep(tc.tile_pool(name="vsil", bufs=2))
        self.ve_pool = ep(tc.tile_pool(name="ve", bufs=2))
        self.crow_pool = ep(tc.tile_pool(name="crow", bufs=3))
        self.prod_pool = ep(tc.tile_pool(name="prod", bufs=2))
        self.o_pool = ep(tc.tile_pool(name="o", bufs=2))

        # PSUM: 7 usable banks: proj/transpose 2 + ps_s 2 + cum 3
        self.proj_pool = ep(tc.tile_pool(name="pj", bufs=2, space="PSUM"))
        self.ss_pool = ep(tc.tile_pool(name="ss", bufs=2, space="PSUM"))
        self.pc_pool = ep(tc.tile_pool(name="pc", bufs=3, space="PSUM"))
        self.pt_pool = self.proj_pool

        # per-batch live state
        self.xt8 = [None] * B_LOC
        self.xbf = [None] * B_LOC
        self.ps_s = [None] * B_LOC
        self.eT = [None] * B_LOC
        self.rdenT = [None] * B_LOC
        self.e_c = [[None] * NT for _ in range(B_LOC)]
        self.r_c = [[None] * NT for _ in range(B_LOC)]
        self.crow = [[None] * NT for _ in range(B_LOC)]
        self.o_all = [None] * B_LOC
        self.s_mm_queue = []  # delayed s-matmuls: (b, tc5, g, ksil)

    # ---- input loads ----
    def load_weights(self):
        nc = self.nc
        self.wk8 = self.const.tile([P, KT, HD], FP8, tag="wk8")
        # two halves on two queues so the first K group starts ASAP
        nc.sync.dma_start(self.wk8[:, 0:KT // 2, :], self.wk8_d[:, 0:KT // 2, :])
        nc.scalar.dma_start(self.wk8[:, KT // 2:KT, :], self.wk8_d[:, KT // 2:KT, :])
        self.load_x(0, split=True)
        self.qbt = self.const.tile([P, NG, H], BF16, tag="qbt")
        nc.scalar.dma_start(self.qbt[:], self.qbt_d[:])
        self.u_sb = self.const.tile([P, P], BF16, tag="u")
        nc.scalar.dma_start(self.u_sb[:], self.u_d[:])
        self.ones_sb = self.const.tile([1, P], F32R, tag="ones")
        nc.scalar.dma_start(self.ones_sb[:], self.ones_d[:])
        self.idb = self.const.tile([H, H], BF16, tag="idb")
        nc.scalar.dma_start(self.idb[:], self.idb_d[:])
        self.idf = self.const.tile([H, H], F32, tag="idf")
        nc.scalar.dma_start(self.idf[:], self.idf_d[:])
        self.wvb = self.const.tile([P, KT, HD], BF16, tag="wvb")
        nc.scalar.dma_start(self.wvb[:], self.wvb_d[:])
        self.wv8 = self.const.tile([P, KT, HD], FP8, tag="wv8")
        nc.sync.dma_start(self.wv8[:], self.wv8_d[:])
        for b in range(1, B_LOC):
            self.load_x(b)

    def load_x(self, b, split=False):
        nc = self.nc
        t = self.xt_pool.tile([P, KT, T], FP8, tag="xt8")
        if split:
            nc.sync.dma_start(t[:, :, 0:512], self.xt8_d[b, :, :, 0:512])
            nc.sync.dma_start(t[:, :, 512:T], self.xt8_d[b, :, :, 512:T])
        else:
            nc.sync.dma_start(t[:], self.xt8_d[b])
        self.xt8[b] = t
        tb = self.xb_pool.tile([P, KT, P], BF16, tag="xbf")
        nc.sync.dma_start(tb[:], self.xbf_d[b])
        self.xbf[b] = tb

    # ---- K path ----
    def emit_k_group(self, b, tc5, g):
        """fp8 DoubleRow projection group + silu; s-matmul is queued (1-delay)."""
        nc = self.nc
        if self.ps_s[b] is None:
            self.ps_s[b] = [
                self.ss_pool.tile([H, 512], F32, tag="ss", name=f"ss{i}")
                for i in range(TC5)
            ]
        pk = self.proj_pool.tile([P, 512], F32, tag="proj")
        for kk in range(KT // 2):
            nc.tensor.matmul(
                pk[:],
                self.wk8[:, 2 * kk:2 * kk + 2, g * P:(g + 1) * P],
                self.xt8[b][:, 2 * kk:2 * kk + 2, tc5 * 512:(tc5 + 1) * 512],
                start=(kk == 0),
                stop=(kk == KT // 2 - 1),
                perf_mode=DR,
            )
        ksil = self.ksil_pool.tile([P, 512], BF16, tag="ksil")
        nc.scalar.activation(ksil[:], pk[:], AF.Silu, scale=1.0 / SC)
        self.s_mm_queue.append((b, tc5, g, ksil))
        if len(self.s_mm_queue) > 1:
            self.flush_s_mm(1)

    def flush_s_mm(self, keep=0):
        nc = self.nc
        while len(self.s_mm_queue) > keep:
            b, tc5, g, ksil = self.s_mm_queue.pop(0)
            nc.tensor.matmul(
                self.ps_s[b][tc5][:],
                self.qbt[:, g, :],
                ksil[:],
                start=(g == 0),
                stop=(g == NG - 1),
            )

    def emit_k_post(self, b):
        """exp -> den scan chain -> rotated reciprocal (ACT + DVE only)."""
        nc = self.nc
        eT = self.et_pool.tile([H, T], BF16, tag="et")
        for tc5 in range(TC5):
            nc.scalar.activation(
                eT[:, tc5 * 512:(tc5 + 1) * 512], self.ps_s[b][tc5][:], AF.Exp
            )
        denT = self.dn_pool.tile([H, T], F32, tag="dn")
        half = T // 2
        nc.vector.tensor_tensor_scan(
            denT[:, 0:half], eT[:, 0:half], eT[:, 0:half], 0.0,
            op0=mybir.AluOpType.add, op1=mybir.AluOpType.bypass,
        )
        nc.vector.tensor_tensor_scan(
            denT[:, half:T], eT[:, half:T], eT[:, half:T],
            denT[:, half - 1:half],
            op0=mybir.AluOpType.add, op1=mybir.AluOpType.bypass,
        )
        # 1/den into a left-padded tile: rdenT[:, 1+t] = 1/den_t. The +1 pad
        # lets the per-chunk transpose read cols [c*P .. c*P+127] so output row
        # m lands on 1/den at t=c*P+m-1 (the rotated cumsum layout); row 0 is
        # patched by a 1-column transpose of t=c*P+127.
        rdenT = self.rd_pool.tile([H, 1 + T], F32, tag="rd")
        nc.vector.memset(rdenT[:, 0:1], 1.0)
        nc.vector.reciprocal(rdenT[:, 1:1 + T], denT[:])
        self.eT[b] = eT
        self.rdenT[b] = rdenT
        self.ps_s[b] = None

    def emit_transpose_pair(self, b, c):
        """Transpose e^T / rden^T for chunks c and c+1 (paired per PSUM tile)."""
        nc = self.nc
        pt_e = self.pt_pool.tile([P, 2, H], BF16, tag="proj")
        for j in range(2):
            nc.tensor.transpose(
                pt_e[:, j, :], self.eT[b][:, (c + j) * P:(c + j + 1) * P], self.idb[:]
            )
        ec = self.ec_pool.tile([P, 2, H], BF16, tag="ec")
        nc.vector.tensor_copy(ec[:], pt_e[:])
        pt_d = self.pt_pool.tile([P, 2, H], F32, tag="proj")
        for j in range(2):
            cc = c + j
            nc.tensor.transpose(
                pt_d[:, j, :], self.rdenT[b][:, cc * P:cc * P + P], self.idf[:]
            )
            nc.tensor.transpose(
                pt_d[0:1, j, :],
                self.rdenT[b][:, 1 + cc * P + P - 1:1 + cc * P + P],
                self.idf[:],
            )
        rc = self.rc_pool.tile([P, 2, H], F32, tag="rc")
        nc.vector.tensor_copy(rc[:], pt_d[:])
        for j in range(2):
            self.e_c[b][c + j] = ec[:, j, :]
            self.r_c[b][c + j] = rc[:, j, :]

    # ---- V path ----
    def emit_v_proj(self, b, c):
        nc = self.nc
        vsil = self.vsil_pool.tile([P, HD], BF16, tag="vsil")
        for nb in range(NB):
            pv = self.proj_pool.tile([P, 512], F32, tag="proj")
            if c == 0:
                # bf16 chunk: fp8 noise on the first time chunk would exceed
                # tolerance (few terms averaged in the streaming softmax yet)
                for k in range(KT):
                    nc.tensor.matmul(
                        pv[:],
                        self.xbf[b][:, k, :],
                        self.wvb[:, k, nb * 512:(nb + 1) * 512],
                        start=(k == 0),
                        stop=(k == KT - 1),
                    )
                nc.scalar.activation(vsil[:, nb * 512:(nb + 1) * 512], pv[:], AF.Silu)
            else:
                for kk in range(KT // 2):
                    nc.tensor.matmul(
                        pv[:],
                        self.xt8[b][:, 2 * kk:2 * kk + 2, c * P:(c + 1) * P],
                        self.wv8[:, 2 * kk:2 * kk + 2, nb * 512:(nb + 1) * 512],
                        start=(kk == 0),
                        stop=(kk == KT // 2 - 1),
                        perf_mode=DR,
                    )
                nc.scalar.activation(
                    vsil[:, nb * 512:(nb + 1) * 512], pv[:], AF.Silu, scale=1.0 / SC
                )
        return vsil

    def emit_v_tail(self, b, c, vsil):
        nc = self.nc
        # ve = v * e (d-major: [p, d, h]; e broadcast over d) on GpSimd
        ve = self.ve_pool.tile([P, HD], BF16, tag="ve")
        e_bc = self.e_c[b][c].unsqueeze(1).broadcast_to((P, D // NB, H))
        # nb=0 half on DVE (feeds the first cum matmul fast), nb=1 on GpSimd
        nc.vector.tensor_mul(
            ve[:, 0:512].rearrange("p (d h) -> p d h", h=H),
            vsil[:, 0:512].rearrange("p (d h) -> p d h", h=H),
            e_bc,
        )
        nc.gpsimd.tensor_mul(
            ve[:, 512:HD].rearrange("p (d h) -> p d h", h=H),
            vsil[:, 512:HD].rearrange("p (d h) -> p d h", h=H),
            e_bc,
        )
        # rotated running cumsum over t via triangular-ones matmul + K=1 carry
        pcs = []
        for nb in range(NB):
            pc = self.pc_pool.tile([P, 512], F32, tag="pc")
            nc.tensor.matmul(
                pc[:], self.u_sb[:], ve[:, nb * 512:(nb + 1) * 512],
                start=True, stop=(c == 0),
            )
            if c > 0:
                nc.tensor.matmul(
                    pc[:], self.ones_sb[:],
                    self.crow[b][c - 1][:, nb, :],
                    start=False, stop=True,
                )
            pcs.append(pc)
        crow = self.crow_pool.tile([1, NB, 512], F32R, tag="crow")
        for nb in range(NB):
            nc.vector.tensor_copy(crow[:, nb, :], pcs[nb][0:1, :])
        self.crow[b][c] = crow
        # prod = cum * rden (read cum straight from PSUM), then head-sum
        if self.o_all[b] is None:
            self.o_all[b] = self.o_pool.tile([P, NT, D], F32, tag="o", name="o")
        prod = self.prod_pool.tile([P, HD], F32, tag="prod")
        r_bc = self.r_c[b][c].unsqueeze(1).broadcast_to((P, D // NB, H))
        for nb in range(NB):
            nc.vector.tensor_mul(
                prod[:, nb * 512:(nb + 1) * 512].rearrange("p (d h) -> p d h", h=H),
                pcs[nb][:].rearrange("p (d h) -> p d h", h=H),
                r_bc,
            )
        nc.vector.reduce_sum(
            self.o_all[b][:, c, :],
            prod[:].rearrange("p (d h) -> p d h", h=H),
            axis=mybir.AxisListType.X,
        )

    def emit_store(self, b):
        nc = self.nc
        o = self.o_all[b]
        dst = self.out_d[b].rearrange("(c m) d -> m c d", m=P)
        nc.sync.dma_start(dst[0:P - 1], o[1:P])
        nc.sync.dma_start(dst[P - 1:P], o[0:1])
        self.o_all[b] = None

    # ---- top level ----
    def build(self):
        self.load_weights()
        # batch 0 K phase (prologue, nothing to interleave into)
        for tc5 in range(TC5):
            for g in range(NG):
                self.emit_k_group(0, tc5, g)
        self.flush_s_mm()
        self.emit_k_post(0)

        for b in range(B_LOC):
            kq = (
                [(tc5, g) for tc5 in range(TC5) for g in range(NG)]
                if b + 1 < B_LOC else []
            )
            for c in range(NT):
                vsil = self.emit_v_proj(b, c)
                # next batch's K groups BEFORE the transposes: a late den
                # transpose (waits on the scan chain) must not block them
                for _ in range(2):
                    if kq:
                        tc5, g = kq.pop(0)
                        self.emit_k_group(b + 1, tc5, g)
                if c < 4:
                    self.emit_transpose_pair(b, 2 * c)
                self.emit_v_tail(b, c, vsil)
            if b + 1 < B_LOC:
                self.flush_s_mm()
                self.emit_k_post(b + 1)
            self.emit_store(b)


def _build(nc, tc, ctx):
    _Builder(nc, tc, ctx).build()


_NC_CACHE = []


def _build_nc():
    if _NC_CACHE:
        return _NC_CACHE[0]
    _apply_tile_patches()
    nc = bass.Bass(trn_type="TRN2", target_bir_lowering=False, debug=False)
    with tile.TileContext(nc) as tc:
        with ExitStack() as ctx:
            _build(nc, tc, ctx)
    _NC_CACHE.append(nc)
    return nc


def _host_prep(x_shard, shared):
    # xt8[b, p, k, t] = x[b, t, k*128+p] as fp8
    xt = np.ascontiguousarray(x_shard.transpose(0, 2, 1))  # [B_loc, I, T]
    xt8 = xt.reshape(B_LOC, KT, P, T).transpose(0, 2, 1, 3)  # [B_loc, P, KT, T]
    m = dict(shared)
    m["xt8"] = np.ascontiguousarray(xt8).astype(ml_dtypes.float8_e4m3fn)
    m["xbf"] = np.ascontiguousarray(xt8[:, :, :, 0:P]).astype(ml_dtypes.bfloat16)
    return m


def kernel(x, kv_kernel, q_kernel):
    x = np.asarray(x, dtype=np.float32)
    kv_kernel = np.asarray(kv_kernel, dtype=np.float32)
    q_kernel = np.asarray(q_kernel, dtype=np.float32)

    wk = kv_kernel[..., 0].reshape(I_DIM, HD)
    wv = kv_kernel[..., 1].reshape(I_DIM, HD)
    # d-major column order for the V path (head-sum becomes stride-1 reduce)
    wv_dm = wv.reshape(I_DIM, H, D).transpose(0, 2, 1).reshape(I_DIM, HD)

    def to_ktile(w):  # [I, HD] -> [P, KT, HD]
        return np.ascontiguousarray(w.reshape(KT, P, HD).transpose(1, 0, 2))

    qbt = np.zeros((P, NG, H), dtype=np.float32)
    for h in range(H):
        g, r = divmod(h * D, P)
        qbt[r:r + D, g, h] = q_kernel[h]
    u = np.triu(np.ones((P, P), dtype=np.float32), k=1)
    u[:, 0] = 1.0
    shared = {
        "wk8": to_ktile(wk * SC).astype(ml_dtypes.float8_e4m3fn),
        "wv8": to_ktile(wv_dm * SC).astype(ml_dtypes.float8_e4m3fn),
        "wvb": to_ktile(wv_dm).astype(ml_dtypes.bfloat16),
        "qbt": qbt.astype(ml_dtypes.bfloat16),
        "u": u.astype(ml_dtypes.bfloat16),
        "ones": np.ones((1, P), dtype=np.float32),
        "idb": np.eye(H, dtype=np.float32).astype(ml_dtypes.bfloat16),
        "idf": np.eye(H, dtype=np.float32),
    }

    nc = _build_nc()
    in_maps = [
        _host_prep(x[c * B_LOC:(c + 1) * B_LOC], shared)
        for c in range(N_CORES)
    ]
    res = bass_utils.run_bass_kernel_spmd(nc, in_maps, core_ids=list(range(N_CORES)))
    out = np.concatenate([r["out"] for r in res.results], axis=0)
    return out.astype(np.float32)
